# revision 1
# baseline (speedup 1.0000x reference)
"""Multi-head attention (B=2, T=2048, D=1024, H=16, causal) on 8 Trainium2
NeuronCores.

Sharding: core c handles batch b = c//4 and head group g = c%4 (4 heads =
256 channels). Wq/Wk/Wv are column-parallel, Wo row-parallel; each core
produces a partial [T, D] output and the host sums the 4 partials per batch
(the "all-reduce") and adds bo.

Per-core kernel (all matmuls in float32r = full-speed fp32 on the PE):
  - Q^T, K^T projected directly into [128, pair, T] transposed layout
    (partition rows = 2 heads x 64 channels) from host-side-transposed
    x^T inputs.  x^T chunks stream through SBUF.
  - V projected into normal [T, d'] layout, with the bias matmul also
    writing a constant 1.0 column per head (vh_aug), so the PV matmul
    computes the softmax denominator for free.
  - Scores computed transposed: S^T[tk, tq] = kh^T . qh^T per 128x512
    tile; exp on ScalarE (scale=1/8 folded in); causal mask applied only
    on diagonal tiles via a precomputed 0/1 mask multiply.
  - PV: x^T_unnorm[65, 512] += vh_aug^T @ expS^T, accumulated over tk
    tiles; row 64 is the denominator.  Normalization via DVE reciprocal +
    a K=1 ones-matmul partition broadcast + DVE multiply.
  - Wo: out[t, :] = sum_k X^T[:,k,t-tile].T @ Wo_rows, copied to SBUF and
    DMA'd out.
"""

import numpy as np
from contextlib import ExitStack

import concourse.bass as bass
import concourse.tile as tile
from concourse import bacc, mybir
from concourse.bass_utils import run_bass_kernel_spmd

F32 = mybir.dt.float32
F32R = mybir.dt.float32r
EXP = mybir.ActivationFunctionType.Exp
MULT = mybir.AluOpType.mult
ADD = mybir.AluOpType.add

B, T, D, H = 2, 2048, 1024, 16
DH = D // H          # 64
HPC = H // 4         # 4 heads per core
DC = DH * HPC        # 256 channels per core
NBLK = T // 512      # 4 Tq blocks of 512
NT128 = T // 128     # 16 T tiles of 128
NCHUNK = D // 128    # 8 contraction chunks

_PROG = None


def _build_program():
    nc = bacc.Bacc("TRN2", target_bir_lowering=False, debug=False)

    xqT = nc.declare_dram_parameter("xqT", [D, T], F32R, isOutput=False)
    xkT = nc.declare_dram_parameter("xkT", [D, T], F32R, isOutput=False)
    xvT = nc.declare_dram_parameter("xvT", [D, T], F32R, isOutput=False)
    wq = nc.declare_dram_parameter("wq", [128, NCHUNK, DC], F32R, isOutput=False)
    wk = nc.declare_dram_parameter("wk", [128, NCHUNK, DC], F32R, isOutput=False)
    wv = nc.declare_dram_parameter("wv", [128, NCHUNK, DC], F32R, isOutput=False)
    wo = nc.declare_dram_parameter("wo", [128, 2, D], F32R, isOutput=False)
    bq = nc.declare_dram_parameter("bq", [128, 2], F32, isOutput=False)
    bk = nc.declare_dram_parameter("bk", [128, 2], F32, isOutput=False)
    bvaug = nc.declare_dram_parameter("bvaug", [1, DC + 4], F32R, isOutput=False)
    onesp = nc.declare_dram_parameter("onesp", [1, 128], F32R, isOutput=False)
    maskp = nc.declare_dram_parameter("maskp", [128, 4, 512], F32, isOutput=False)
    outp = nc.declare_dram_parameter("outp", [T, D], F32, isOutput=True)

    with tile.TileContext(nc) as tc, ExitStack() as ctx:
        cpool = ctx.enter_context(tc.tile_pool(name="consts", bufs=1))
        persist = ctx.enter_context(tc.tile_pool(name="persist", bufs=1))
        xp = ctx.enter_context(tc.tile_pool(name="xchunks", bufs=6))
        esp = ctx.enter_context(tc.tile_pool(name="es", bufs=4))
        smp = ctx.enter_context(tc.tile_pool(name="small", bufs=2))
        sop = ctx.enter_context(tc.tile_pool(name="so", bufs=3))
        pp = ctx.enter_context(tc.tile_pool(name="pp", bufs=2, space="PSUM"))
        stp = ctx.enter_context(tc.tile_pool(name="stp", bufs=2, space="PSUM"))
        xup = ctx.enter_context(tc.tile_pool(name="xup", bufs=2, space="PSUM"))

        # ---- constants ----
        wq_sb = cpool.tile([128, NCHUNK, DC], F32R)
        nc.scalar.dma_start(wq_sb[:], wq[:])
        wk_sb = cpool.tile([128, NCHUNK, DC], F32R)
        nc.scalar.dma_start(wk_sb[:], wk[:])
        wv_sb = cpool.tile([128, NCHUNK, DC], F32R)
        nc.scalar.dma_start(wv_sb[:], wv[:])
        wo_sb = cpool.tile([128, 2, D], F32R)
        nc.scalar.dma_start(wo_sb[:], wo[:])
        bq_sb = cpool.tile([128, 2], F32)
        nc.scalar.dma_start(bq_sb[:], bq[:])
        bk_sb = cpool.tile([128, 2], F32)
        nc.scalar.dma_start(bk_sb[:], bk[:])
        bvaug_sb = cpool.tile([1, DC + 4], F32R)
        nc.scalar.dma_start(bvaug_sb[:], bvaug[:])
        ones_sb = cpool.tile([1, 128], F32R)
        nc.scalar.dma_start(ones_sb[:], onesp[:])
        mask_sb = cpool.tile([128, 4, 512], F32)
        nc.scalar.dma_start(mask_sb[:], maskp[:])

        # persistent activations
        QT = persist.tile([128, 2, T], F32R)      # [2 heads x 64, pair, T]
        KT = persist.tile([128, 2, T], F32R)
        Vaug = persist.tile([128, NT128, HPC, DH + 1], F32R)
        XT = persist.tile([128, 2, T], F32R)      # attention out, transposed

        # preload the exp table set early (one-time ~2.7us)
        warm = smp.tile([1, 2], F32, tag="warm")
        nc.scalar.activation(warm[:], ones_sb[0:1, 0:2], EXP, scale=1.0)

        # ---- K and Q projections: OUT[., pair, tq] += W_pair_c^T @ x^T ----
        for xparam, w_sb, b_sb, OUT in (
            (xkT, wk_sb, bk_sb, KT),
            (xqT, wq_sb, bq_sb, QT),
        ):
            for blk in range(NBLK):
                chunks = []
                for c in range(NCHUNK):
                    xc = xp.tile([128, 512], F32R, tag="xkq", name="xc")
                    eng = nc.sync if c % 2 == 0 else nc.gpsimd
                    eng.dma_start(
                        xc[:],
                        xparam[128 * c : 128 * (c + 1), 512 * blk : 512 * (blk + 1)],
                    )
                    chunks.append(xc)
                ps0 = pp.tile([128, 512], F32, tag="proj", name="ps0")
                ps1 = pp.tile([128, 512], F32, tag="proj", name="ps1")
                for c in range(NCHUNK):
                    nc.tensor.matmul(
                        ps0[:], w_sb[:, c, 0:128], chunks[c][:],
                        start=(c == 0), stop=(c == NCHUNK - 1),
                        skip_group_check=True,
                    )
                    nc.tensor.matmul(
                        ps1[:], w_sb[:, c, 128:256], chunks[c][:],
                        start=(c == 0), stop=(c == NCHUNK - 1),
                        skip_group_check=True,
                    )
                for p, pst in ((0, ps0), (1, ps1)):
                    nc.vector.tensor_scalar(
                        OUT[:, p, 512 * blk : 512 * (blk + 1)],
                        pst[:], b_sb[:, p : p + 1], None, op0=ADD,
                    )

        # ---- V projection into Vaug (normal layout + ones column) ----
        for s in range(8):
            vchunks = []
            for c in range(NCHUNK):
                xc = xp.tile([128, 256], F32R, tag="xv", name="xvc")
                eng = nc.sync if c % 2 == 0 else nc.gpsimd
                eng.dma_start(
                    xc[:],
                    xvT[128 * c : 128 * (c + 1), 256 * s : 256 * (s + 1)],
                )
                vchunks.append(xc)
            pv0 = pp.tile([128, DC + 4], F32, tag="proj", name="pv0")
            pv1 = pp.tile([128, DC + 4], F32, tag="proj", name="pv1")
            for c in range(NCHUNK):
                nc.tensor.matmul(
                    pv0[:, 0:DC], vchunks[c][:, 0:128], wv_sb[:, c, :],
                    start=(c == 0), stop=False, skip_group_check=True,
                )
                nc.tensor.matmul(
                    pv1[:, 0:DC], vchunks[c][:, 128:256], wv_sb[:, c, :],
                    start=(c == 0), stop=False, skip_group_check=True,
                )
            # bias matmul: accumulates bv into cols 0:256 and (has_written
            # unset there) writes 1.0 into cols 256:260
            nc.tensor.matmul(
                pv0[:], ones_sb[0:1, 0:128], bvaug_sb[:],
                start=False, stop=True, skip_group_check=True,
            )
            nc.tensor.matmul(
                pv1[:], ones_sb[0:1, 0:128], bvaug_sb[:],
                start=False, stop=True, skip_group_check=True,
            )
            for k, pv in ((0, pv0), (1, pv1)):
                t = 2 * s + k
                nc.vector.tensor_copy(
                    Vaug[:, t, :, 0:DH],
                    pv[:, 0:DC].rearrange("p (h d) -> p h d", h=HPC),
                )
                nc.vector.tensor_copy(
                    Vaug[:, t, :, DH : DH + 1],
                    pv[:, DC : DC + 4].rearrange("p (h d) -> p h d", h=HPC),
                )

        # ---- attention ----
        for p in range(2):
            for i in range(NBLK):
                xu0 = xup.tile([DH + 1, 512], F32, tag="xu", name="xu0")
                xu1 = xup.tile([DH + 1, 512], F32, tag="xu", name="xu1")
                njt = 4 * i + 4
                for j in range(njt):
                    ps_t = stp.tile([128, 2, 512], F32, tag="st", name="ps_t")
                    for hp in range(2):
                        nc.tensor.matmul(
                            ps_t[:, hp, :],
                            KT[64 * hp : 64 * hp + 64, p, 128 * j : 128 * (j + 1)],
                            QT[64 * hp : 64 * hp + 64, p, 512 * i : 512 * (i + 1)],
                            start=True, stop=True, skip_group_check=True,
                        )
                    es = esp.tile([128, 2, 512], F32R, tag="es", name="es")
                    nc.scalar.activation(es[:], ps_t[:], EXP, scale=1.0 / np.sqrt(DH))
                    if j >= 4 * i:
                        J = j - 4 * i
                        for hp in range(2):
                            nc.vector.tensor_tensor(
                                es[:, hp, :], es[:, hp, :], mask_sb[:, J, :], op=MULT
                            )
                    for hp, xu in ((0, xu0), (1, xu1)):
                        nc.tensor.matmul(
                            xu[:], Vaug[:, j, 2 * p + hp, :], es[:, hp, :],
                            start=(j == 0), stop=(j == njt - 1),
                            skip_group_check=True,
                        )
                for hp, xu in ((0, xu0), (1, xu1)):
                    rd = smp.tile([1, 512], F32R, tag="rd", name="rd")
                    with nc.allow_low_precision(reason="fp32r denominators"):
                        nc.vector.reciprocal(rd[:], xu[DH : DH + 1, :])
                    bc = stp.tile([64, 512], F32, tag="st", name="bc")
                    nc.tensor.matmul(
                        bc[:], ones_sb[0:1, 0:64], rd[:],
                        start=True, stop=True, skip_group_check=True,
                    )
                    bcs = smp.tile([64, 512], F32, tag="bcs", name="bcs")
                    nc.vector.tensor_copy(bcs[:], bc[:])
                    nc.vector.tensor_tensor(
                        XT[64 * hp : 64 * hp + 64, p, 512 * i : 512 * (i + 1)],
                        xu[0:DH, :], bcs[:], op=MULT,
                    )

        # ---- output projection (row-parallel partial) ----
        for t in range(NT128):
            for n in range(2):
                po = pp.tile([128, 512], F32, tag="proj", name="po")
                nc.tensor.matmul(
                    po[:], XT[:, 0, 128 * t : 128 * (t + 1)],
                    wo_sb[:, 0, 512 * n : 512 * (n + 1)],
                    start=True, stop=False, skip_group_check=True,
                )
                nc.tensor.matmul(
                    po[:], XT[:, 1, 128 * t : 128 * (t + 1)],
                    wo_sb[:, 1, 512 * n : 512 * (n + 1)],
                    start=False, stop=True, skip_group_check=True,
                )
                so = sop.tile([128, 512], F32, tag="so", name="so")
                if (t + n) % 2 == 0:
                    nc.scalar.copy(so[:], po[:])
                else:
                    nc.vector.tensor_copy(so[:], po[:])
                nc.sync.dma_start(
                    outp[128 * t : 128 * (t + 1), 512 * n : 512 * (n + 1)], so[:]
                )

    nc.compile()
    return nc


def _get_program():
    global _PROG
    if _PROG is None:
        _PROG = _build_program()
    return _PROG


def _make_mask():
    r = np.arange(128)[:, None]
    c = np.arange(512)[None, :]
    m = np.zeros((128, 4, 512), np.float32)
    for J in range(4):
        m[:, J, :] = (c >= 128 * J + r).astype(np.float32)
    return m


def _core_inputs(inputs, b, g):
    """Per-core input map (host-side sharding/layout prep)."""
    f = np.float32
    q, k, v = inputs["q"], inputs["k"], inputs["v"]
    sl = slice(DC * g, DC * (g + 1))
    wq = np.ascontiguousarray(
        np.asarray(inputs["Wq"], f)[:, sl].reshape(NCHUNK, 128, DC).transpose(1, 0, 2)
    )
    wk = np.ascontiguousarray(
        np.asarray(inputs["Wk"], f)[:, sl].reshape(NCHUNK, 128, DC).transpose(1, 0, 2)
    )
    wv = np.ascontiguousarray(
        np.asarray(inputs["Wv"], f)[:, sl].reshape(NCHUNK, 128, DC).transpose(1, 0, 2)
    )
    wo = np.ascontiguousarray(
        np.asarray(inputs["Wo"], f)[sl, :].reshape(2, 128, D).transpose(1, 0, 2)
    )
    bq = np.ascontiguousarray(np.asarray(inputs["bq"], f)[sl].reshape(2, 128).T)
    bk = np.ascontiguousarray(np.asarray(inputs["bk"], f)[sl].reshape(2, 128).T)
    bvaug = np.concatenate(
        [np.asarray(inputs["bv"], f)[sl], np.ones(4, f)]
    ).reshape(1, DC + 4)
    return {
        "wq": wq, "wk": wk, "wv": wv, "wo": wo,
        "bq": bq, "bk": bk, "bvaug": bvaug,
        "onesp": np.ones((1, 128), f),
        "maskp": _make_mask(),
    }


def run_cores(inputs, trace=False, trace_cores=None):
    nc = _get_program()
    f = np.float32
    xT = {}
    for b in range(B):
        xT[b] = {
            "xqT": np.ascontiguousarray(np.asarray(inputs["q"], f)[b].T),
            "xkT": np.ascontiguousarray(np.asarray(inputs["k"], f)[b].T),
            "xvT": np.ascontiguousarray(np.asarray(inputs["v"], f)[b].T),
        }
    in_maps = []
    for c in range(8):
        b, g = divmod(c, 4)
        m = _core_inputs(inputs, b, g)
        m.update(xT[b])
        in_maps.append(m)
    kw = {}
    if trace:
        kw = dict(trace=True, trace_cores=trace_cores or [0])
    res = run_bass_kernel_spmd(nc, in_maps, list(range(8)), **kw)
    bo = np.asarray(inputs["bo"], f)
    out = np.empty((B, T, D), f)
    for b in range(B):
        acc = res.results[4 * b]["outp"].astype(f).copy()
        for g in range(1, 4):
            acc += res.results[4 * b + g]["outp"]
        out[b] = acc + bo
    return out, res


def kernel(**inputs) -> np.ndarray:
    out, _ = run_cores(inputs)
    return out
